# revision 51
# baseline (speedup 1.0000x reference)
"""TRN2 Bass kernel for nn_BiAttention (B=48, S=512, H=768) on 8 NeuronCores.

Data-parallel: 6 samples per core, weights replicated.

Per-sample math (matches the reference exactly):
  Q = x @ Wq.T + bq ; K = x @ Wk.T + bk ; V = x @ Wv.T + bv
  scores = Q @ K.T / sqrt(H) + A        (A = segment allow/additive mask)
  attn = softmax(scores, axis=-1)
  out = tanh((attn @ V) @ W0.T + b0) @ W1.T + b1 + x

Kernel design (fused form):
  - Q.K fusion: scores = x @ Bsc @ x.T + u(k) + v(q) + c + A, with
    Bsc = Wq.T@Wk/sqrt(H) precomputed on the host, and the bias
    cross-terms u = x@(Wk.T bq)/sqrt(H), v = x@(Wq.T bk)/sqrt(H),
    c = bq.bk/sqrt(H) computed per-sample on the host. Only ONE device
    projection (P1T = Bsc.T @ xT) instead of Q and K.
  - V.W0 fusion: (attn@V)@W0.T + b0 = attn@(x@Wc.T) + b0' with
    Wc = W0@Wv and b0' = b0 + W0@bv (uses sum(attn_row)=1). FC0
    disappears; tanh applies directly to the PV psum.
  - scores computed TRANSPOSED (sT[k,q]) so the attention matrix is
    already in the right layout for the P@V matmul — no transposes.
  - The additive mask + u/v/c terms are rank-4: host packs l4=[rowQ*64,
    rowC*64, u+c, 1] (k-side) and r4=[isq, 1-isq, 1, v] (q-side); one
    K=4 bf16 matmul accumulates all of it into the scores psum.
  - softmax without max-subtraction (scores are O(1); -1e9 -> exp = 0;
    P1T is stored x64 so exp uses scale=1/64). Column sums via a
    ones^T DoubleRow matmul; 1/Z broadcast back with a K=1 matmul;
    expT normalized in place.
  - All heavy matmuls are fp8e4m3 DoubleRow (2 weights/PE cell):
    P1T/Vc/FC1 contract h-pairs, scores contract o-pairs, PV contracts
    k-pairs. Host prescales: Bsc x8192, Wc x256, W1 x256 (fp8e4m3
    subnormal floor is ~2^-9), descaled in psum->sbuf copies / on host.
  - b1 + x + the FC1 1/256 descale are applied on the HOST in fp32;
    the device ships the raw FC1 psum.
  - psum->sbuf copies alternate DVE/ACT (balanced), and bias-free
    copies are pair-merged via 2-bank psum tiles to halve op overhead.
"""

import numpy as np
import ml_dtypes

B, S, H = 48, 512, 768
NCORES = 8
BPC = B // NCORES  # samples per core
P = 128
HC = H // P   # 6 chunks of 128 over hidden dim
HJ = HC // 2  # 3 DoubleRow pair-groups over hidden dim
SC = S // P   # 4 chunks of 128 over sequence dim
SJ = SC // 2  # 2 DoubleRow pair-groups over sequence dim
NEG = -16384.0  # e5m2-exact; exp((s-16384)/64) underflows to exactly 0
RS = float(1.0 / np.sqrt(np.float32(H)))
WS_B = 8192.0   # Bsc prescale
SB_OUT = 64.0   # P1T storage scale (exp compensates with scale=1/64)
WS_C = 256.0    # Wc prescale
WS_1 = 256.0    # W1 prescale

_cache = {}
_STAGE_MARKS = []  # (inst_id_watermark, label) for timeline attribution


def _build_program(spans=None):
    """spans: per-slot ((4 x (lo,hi) C-chunk q-spans), (2 x (lo,hi) F-pair
    q-spans)) from the conservative divide_pos ranges of the samples that
    share each slot across the 8 cores (see kernel()). None = full width.

    Restriction is an optimization only -- correctness never depends on it:
    expT is exactly 0 on disallowed (k,q) pairs (the full-width rank-4 mask
    matmul puts NEG there), so any matmul over a WIDER q-range just adds
    zeros. Spans must only be conservative (cover all allowed pairs)."""
    import concourse.bass as bass
    import concourse.mybir as mybir
    import concourse.tile as tile
    from concourse import bacc

    if spans is None:
        spans = tuple((((0, S),) * SC, ((0, S),) * SJ) for _ in range(BPC))

    # Per-slot pair geometry. pair p covers k-chunks (2p, 2p+1); its union
    # span U_p is the q-range where ANY of its keys may attend. The et tile
    # is only valid inside U_p (exp is restricted to it), so every reader
    # (zsum/norm/PV) is restricted identically. A full-width pair (the
    # "mixed" one) always exists except when every d == 256 (disjoint
    # spans); orderings put the full pair first so start=True matmuls
    # initialize every column that full-width readers (tanh) touch.
    slot_geom = []
    for b_ in range(BPC):
        cs, fs = spans[b_]
        full = [p_ for p_ in range(SJ) if fs[p_] == (0, S)]
        slot_geom.append({
            "cs": cs, "U": fs,
            "order": [full[0], 1 - full[0]] if full else [0, 1],
            "disjoint": not full,
        })

    f32 = mybir.dt.float32
    bf16 = mybir.dt.bfloat16
    f8 = mybir.dt.float8e4
    f85 = mybir.dt.float8e5
    AF = mybir.ActivationFunctionType
    ALU = mybir.AluOpType
    DR = mybir.MatmulPerfMode.DoubleRow

    nc = bacc.Bacc("TRN2", target_bir_lowering=False, debug=False)

    # ---- DRAM tensors (per-core) ----
    xT_d = nc.dram_tensor("xT", [BPC, H, S], f8, kind="ExternalInput")
    w_d = {
        name: nc.dram_tensor(name, [H, H], f8, kind="ExternalInput")
        for name in ["Bsc", "WcT", "W1T"]
    }
    b0p_d = nc.dram_tensor("b0p", [H], f32, kind="ExternalInput")
    # [2, 4, S]: l4 and r4 packed in one tensor (halves the mask DMAs):
    # [:, 0:2, :] = l4, [:, 2:4, :] = r4; 2 partitions x 2 DoubleRow pairs
    m4_d = nc.dram_tensor("m4", [BPC, 2, 4, S], f85, kind="ExternalInput")
    outT_d = nc.dram_tensor("outT", [BPC, H, S], bf16, kind="ExternalOutput")

    with tile.TileContext(nc) as tc:
        with (
            tc.tile_pool(name="wpool", bufs=1) as wpool,
            tc.tile_pool(name="xpool", bufs=4) as xpool,
            tc.tile_pool(name="mpool", bufs=4) as mpool,
            tc.tile_pool(name="ppool", bufs=2) as ppool,
            tc.tile_pool(name="vpool", bufs=3) as vpool,
            tc.tile_pool(name="epool", bufs=2) as epool,
            tc.tile_pool(name="rpool", bufs=2) as rpool,
            tc.tile_pool(name="opool", bufs=2) as opool,
            tc.tile_pool(name="zpool", bufs=2) as zpool,
            tc.tile_pool(name="psum2", bufs=4, space="PSUM") as psum2,
        ):
            # --- DVE/ACT/Pool copy balancer: psum -> sbuf (in*scale + bias)
            # Tracks estimated busy-ns per engine (cost-model rates:
            # DVE 1.042 ns/col + ~250 fixed, ACT 0.833 + ~240,
            # Pool 1.39 (0.6 gpsimd efficiency) + ~90) and routes each
            # flexible copy to the engine with the smallest backlog.
            eng_state = {"dve": 0.0, "act": 0.0, "pool": 0.0}
            RATE = {"dve": 1.042, "act": 0.833, "pool": 1.39}
            FIX = {"dve": 250.0, "act": 240.0, "pool": 90.0}

            def charge(eng, cols, fixed=None):
                eng_state[eng] += cols * RATE[eng] + (
                    FIX[eng] if fixed is None else fixed)

            def copy_out(dst, src, scale=None, bias=None, force=None):
                # GPSIMD/Pool cannot access PSUM on TRN2 hardware, so
                # psum->sbuf copies only balance across DVE and ACT.
                cols = 1
                for d in dst.shape[1:]:
                    cols *= d
                eng = force
                if eng is None:
                    eng = ("dve" if eng_state["dve"] + cols * RATE["dve"]
                           <= eng_state["act"] + cols * RATE["act"] else "act")
                charge(eng, cols)
                if eng == "dve" or eng == "pool":
                    e = nc.vector if eng == "dve" else nc.gpsimd
                    if bias is None and scale is None:
                        e.tensor_copy(dst, src)
                    elif bias is None:
                        e.tensor_scalar_mul(dst, src, scale)
                    elif scale is None:
                        e.tensor_scalar(
                            dst, src, scalar1=bias, scalar2=None, op0=ALU.add
                        )
                    else:
                        e.tensor_scalar(
                            dst, src, scalar1=scale, scalar2=bias,
                            op0=ALU.mult, op1=ALU.add,
                        )
                else:
                    nc.scalar.activation(
                        dst, src, func=AF.Identity,
                        bias=0.0 if bias is None else bias,
                        scale=1.0 if scale is None else scale,
                    )

            def load_sample(b, split_x=False):
                x_t = xpool.tile([P, HC, S], f8, tag="xT")
                xr = xT_d.ap()[b].rearrange("(c p) s -> p c s", p=P)
                if split_x:
                    nc.sync.dma_start(x_t[:, : HC // 2, :], xr[:, : HC // 2, :])
                    nc.sync.dma_start(x_t[:, HC // 2:, :], xr[:, HC // 2:, :])
                else:
                    nc.sync.dma_start(x_t[:], xr)
                m4 = mpool.tile([2, 4, S], f85, tag="m4")
                nc.sync.dma_start(m4[:], m4_d.ap()[b])
                return x_t, m4[:, 0:2, :], m4[:, 2:4, :]

            w_sb = {}

            # weights go down the gpsimd (SWDGE) queue, in parallel with the
            # sample-input DMAs on the sync (HWDGE) queue. SWDGE descriptor
            # generation runs ON the Pool engine (~1.2us per weight DMA), so
            # charge the balancer to keep early copies off Pool.
            def load_w(name, split=False):
                t = wpool.tile([P, HC, H], f8, tag=name)
                wr = w_d[name].ap().rearrange("(c p) o -> p c o", p=P)
                if split:
                    nc.gpsimd.dma_start(t[:, : HC // 2, :], wr[:, : HC // 2, :])
                    nc.gpsimd.dma_start(t[:, HC // 2:, :], wr[:, HC // 2:, :])
                    charge("pool", 0, fixed=2 * (994 + 0.34 * 384))
                else:
                    nc.gpsimd.dma_start(t[:], wr)
                    charge("pool", 0, fixed=994 + 0.34 * 768)
                w_sb[name] = t

            # startup: x(0) pairs stream on sync/HWDGE while Bsc pairs
            # stream in parallel on gpsimd/SWDGE -- the j-outer A stage
            # consumes pair j as it lands. WcT rides sync right after x(0)
            # (ready ~when B0 starts); W1T/b0p trail on gpsimd (needed two
            # iterations later).
            x0 = xpool.tile([P, HC, S], f8, tag="xT")
            x0r = xT_d.ap()[0].rearrange("(c p) s -> p c s", p=P)
            bsc_t = wpool.tile([P, HC, H], f8, tag="Bsc")
            bsc_r = w_d["Bsc"].ap().rearrange("(c p) o -> p c o", p=P)
            w_sb["Bsc"] = bsc_t
            for j in range(HJ):
                sl = slice(2 * j, 2 * j + 2)
                nc.sync.dma_start(x0[:, sl, :], x0r[:, sl, :])
                nc.gpsimd.dma_start(bsc_t[:, sl, :], bsc_r[:, sl, :])
                charge("pool", 0, fixed=994 + 0.34 * 256)
            wct_t = wpool.tile([P, HC, H], f8, tag="WcT")
            nc.sync.dma_start(
                wct_t[:], w_d["WcT"].ap().rearrange("(c p) o -> p c o", p=P))
            w_sb["WcT"] = wct_t
            m40 = mpool.tile([2, 4, S], f85, tag="m4")
            nc.sync.dma_start(m40[:], m4_d.ap()[0])
            sample0 = (x0, m40[:, 0:2, :], m40[:, 2:4, :])

            load_w("W1T")
            b0p_sb = wpool.tile([P, HC], f32, tag="b0p")
            nc.gpsimd.dma_start(
                b0p_sb[:], b0p_d.ap().rearrange("(c p) -> p c", p=P)
            )
            charge("pool", 0, fixed=994)
            # [P, 2, 128]: zsum lhsT with free-size 128 -> every output
            # partition gets the column sum (zsum + broadcast in one matmul)
            ones_k = wpool.tile([P, 2, P], f8, tag="ones_k")
            nc.vector.memset(ones_k, 1.0)

            def proj_dr(wname, rhs_tile, o, ps, n=S):
                """accumulate one o-chunk of W.T@rhs with DoubleRow fp8"""
                for j in range(HJ):
                    nc.tensor.matmul(
                        ps[:, :n],
                        lhsT=w_sb[wname][:, 2 * j:2 * j + 2, o * P:(o + 1) * P],
                        rhs=rhs_tile[:, 2 * j:2 * j + 2, :n],
                        start=(j == 0),
                        stop=(j == HJ - 1),
                        perf_mode=DR,
                    )

            def stage_a(b, loaded, j_outer=False):
                """P1T[h', q] = (Bsc.T @ xT), stored fp8 at x64 scale:
                returns (p1 tile, [3 group thunks]).
                j_outer=True (sample 0 only): contraction-pair-major order
                across three live psum tiles, so the first matmul wave
                needs only the first x/Bsc pair DMA and PE fills as the
                startup DMAs land, pair by pair."""
                x_t, m_l4, m_r4 = loaded
                p1 = ppool.tile([P, HC, S], f8, tag="P1T")

                def group(jo):
                    ps = psum2.tile([P, 2, S], f32, tag="ps2")
                    for i in range(2):
                        proj_dr("Bsc", x_t, 2 * jo + i, ps[:, i, :])
                    # forced DVE: keeps ACT clear for exp/tanh in the
                    # C..F window so the tanh->G psum-recycle chain is fast
                    copy_out(p1[:, 2 * jo:2 * jo + 2, :], ps[:],
                             scale=float(SB_OUT / WS_B), force="dve")

                def all_groups():
                    pss = []
                    for _k in range(HJ):
                        ps_k = psum2.tile([P, 2, S], f32, tag="ps2",
                                          name=f"ps_a{_k}")
                        pss.append(ps_k)
                    for j in range(HJ):
                        for jo in range(HJ):
                            for i in range(2):
                                o = 2 * jo + i
                                nc.tensor.matmul(
                                    pss[jo][:, i, :],
                                    lhsT=w_sb["Bsc"][:, 2 * j:2 * j + 2,
                                                     o * P:(o + 1) * P],
                                    rhs=x_t[:, 2 * j:2 * j + 2, :],
                                    start=(j == 0), stop=(j == HJ - 1),
                                    perf_mode=DR,
                                )
                    for jo in range(HJ):
                        copy_out(p1[:, 2 * jo:2 * jo + 2, :], pss[jo][:],
                                 scale=float(SB_OUT / WS_B), force="dve")

                if j_outer:
                    return p1, [all_groups, None, None]
                return p1, [lambda jo=jo: group(jo) for jo in range(HJ)]

            def stage_b(b, loaded):
                """Vc[s, o] = x @ Wc.T (PV's lhsT layout): 4 group thunks"""
                x_t, m_l4, m_r4 = loaded
                vc = vpool.tile([P, SC, H], f8, tag="Vc")

                def group(s4):
                    ps = psum2.tile([P, 2, S], f32, tag="ps2")
                    for half in range(2):
                        for j in range(HJ):
                            nc.tensor.matmul(
                                ps[:, half, : H // 2],
                                lhsT=x_t[:, 2 * j:2 * j + 2, s4 * P:(s4 + 1) * P],
                                rhs=w_sb["WcT"][:, 2 * j:2 * j + 2,
                                               half * (H // 2):(half + 1) * (H // 2)],
                                start=(j == 0),
                                stop=(j == HJ - 1),
                                perf_mode=DR,
                            )
                    copy_out(
                        vc[:, s4, :].rearrange("p (i n) -> p i n", i=2),
                        ps[:, :, : H // 2], scale=float(1.0 / WS_C),
                    )

                return vc, [lambda s4=s4: group(s4) for s4 in range(SC)]

            def stage_c(b, loaded, p1):
                """scoresT[k,q]*64 = x.T @ P1T + l4.T @ r4 ; exp(/64).
                The rank-4 mask matmul covers the FULL width (start=True:
                disallowed columns get NEG -> exp 0); the score matmuls
                only run over each k-chunk's conservative q-span."""
                x_t, m_l4, m_r4 = loaded
                geo = slot_geom[b]
                et = epool.tile([P, SC, S], f8, tag="expT")

                def group(jp):
                    # group jp == k-pair jp. psum/et are only defined on the
                    # pair union span U (all readers restrict to it).
                    ulo, uhi = geo["U"][jp]
                    ps = psum2.tile([P, 2, S], f32, tag="ps2")
                    for i in range(2):
                        k4 = 2 * jp + i
                        lo, hi = geo["cs"][k4]
                        nc.tensor.matmul(
                            ps[:, i, ulo:uhi],
                            lhsT=m_l4[:, :, k4 * P:(k4 + 1) * P],
                            rhs=m_r4[:, :, ulo:uhi],
                            start=True, stop=False,
                            perf_mode=DR,
                            skip_group_check=True,
                        )
                        for j in range(HJ):
                            nc.tensor.matmul(
                                ps[:, i, lo:hi],
                                lhsT=x_t[:, 2 * j:2 * j + 2, k4 * P:(k4 + 1) * P],
                                rhs=p1[:, 2 * j:2 * j + 2, lo:hi],
                                start=False, stop=(j == HJ - 1),
                                perf_mode=DR,
                                skip_group_check=True,
                            )
                    charge("act", 2 * (uhi - ulo))
                    nc.scalar.activation(
                        et[:, 2 * jp:2 * jp + 2, ulo:uhi],
                        ps[:, :, ulo:uhi],
                        func=AF.Exp, scale=float(1.0 / SB_OUT),
                    )

                return et, [lambda jp=jp: group(jp) for jp in range(SJ)]

            def zsum(et, b):
                """column sums of expT, broadcast to all 128 partitions:
                ones lhsT with free-size 128 makes every output partition
                the same column sum (fuses zsum + broadcast). Uses half of
                a shared-pool psum tile (all 8 banks belong to psum2).
                Full(mixed)-span pair first; the pure pair only sums its
                union span (its keys contribute 0 elsewhere)."""
                geo = slot_geom[b]
                ps_zt = psum2.tile([P, 2, S], f32, tag="ps2")
                ps_z = ps_zt[:, 0, :]
                for n, j in enumerate(geo["order"]):
                    lo, hi = geo["U"][j]
                    nc.tensor.matmul(
                        ps_z[:, lo:hi],
                        lhsT=ones_k[:],
                        rhs=et[:, 2 * j:2 * j + 2, lo:hi],
                        start=(n == 0 or geo["disjoint"]),
                        stop=(n == SJ - 1 or geo["disjoint"]),
                        perf_mode=DR,
                        skip_group_check=True,
                    )
                return ps_z

            def zb_norm(et, ps_z, b):
                """reciprocal of broadcast sums, normalize expT in place
                (only over each pair's union span -- et is garbage outside
                and never read). The wider span normalizes on DVE; the
                narrower one is sbuf-only work for the idle Pool engine."""
                geo = slot_geom[b]
                rz = zpool.tile([P, S], bf16, tag="rz")
                charge("dve", S)
                with nc.allow_low_precision(reason="1/Z in bf16; expT is fp8"):
                    nc.vector.reciprocal(rz[:], ps_z[:])
                wa, wb = geo["order"]
                # Pool (idle engine) takes the WIDE span; DVE the narrow
                # one. The normalized et isn't needed until next iteration,
                # so Pool's 2.4x slower multiply is off the critical path.
                lo, hi = geo["U"][wa]
                charge("pool", 2 * (hi - lo) * 1.42)  # 0.42 gpsimd Multiply
                nc.gpsimd.tensor_mul(
                    et[:, 2 * wa:2 * wa + 2, lo:hi],
                    et[:, 2 * wa:2 * wa + 2, lo:hi],
                    rz[:, None, lo:hi].to_broadcast((P, 2, hi - lo)))
                lo, hi = geo["U"][wb]
                charge("dve", 2 * (hi - lo))
                nc.vector.tensor_mul(
                    et[:, 2 * wb:2 * wb + 2, lo:hi],
                    et[:, 2 * wb:2 * wb + 2, lo:hi],
                    rz[:, None, lo:hi].to_broadcast((P, 2, hi - lo)))

            def stage_f(b, vc, et):
                """PV + tanh for sample b -> hT (fp8): 3 pair thunks.
                The wider-span k-pair goes first at FULL width (start=True
                initializes every column; expT is 0 outside its span so
                the extra columns just add 0); the second pair is
                restricted to its conservative q-span."""
                geo = slot_geom[b]
                ht = rpool.tile([P, HC, S], f8, tag="hT")

                def group(jo):
                    ps = psum2.tile([P, 2, S], f32, tag="ps2")
                    for n, j in enumerate(geo["order"]):
                        lo, hi = geo["U"][j]
                        for i in range(2):
                            h = 2 * jo + i
                            nc.tensor.matmul(
                                ps[:, i, lo:hi],
                                lhsT=vc[:, 2 * j:2 * j + 2, h * P:(h + 1) * P],
                                rhs=et[:, 2 * j:2 * j + 2, lo:hi],
                                start=(n == 0 or geo["disjoint"]),
                                stop=(n == SJ - 1 or geo["disjoint"]),
                                perf_mode=DR,
                                skip_group_check=True,
                            )
                    for i in range(2):
                        # per-chunk tanh: bias b0' is per-partition per chunk
                        h = 2 * jo + i
                        charge("act", S)
                        nc.scalar.activation(
                            ht[:, h, :], ps[:, i, :], func=AF.Tanh,
                            bias=b0p_sb[:, h:h + 1],
                        )

                return ht, [lambda jo=jo: group(jo) for jo in range(HJ)]

            def stage_g(b, ht, final=False):
                """FC1 (raw psum, x256) + store; host adds b1+x and /256.
                j-major so only the last matmul wave needs the last tanh.
                Stores ride three different DMA queues (SP / ACT / DVE
                HWDGE) so no sequencer serializes them. final=True splits
                each pair copy three ways so the kernel-tail copy+store
                chain is a third as long."""
                ot = opool.tile([P, HC, S], bf16, tag="outT")
                our = outT_d.ap()[b].rearrange("(c p) s -> p c s", p=P)
                # staggered contraction order: group jo's LAST matmul uses
                # ht pair (jo+1)%3, so no group's completion waits on the
                # latest tanh, and the final-stored group (G2) ends on the
                # oldest one.
                jord = [[0, 2, 1], [1, 0, 2], [1, 2, 0]]

                def group(jo):
                    ps = psum2.tile([P, 2, S], f32, tag="ps2")
                    for n, j in enumerate(jord[jo]):
                        for i in range(2):
                            o = 2 * jo + i
                            nc.tensor.matmul(
                                ps[:, i, :],
                                lhsT=w_sb["W1T"][:, 2 * j:2 * j + 2, o * P:(o + 1) * P],
                                rhs=ht[:, 2 * j:2 * j + 2, :],
                                start=(n == 0), stop=(n == HJ - 1),
                                perf_mode=DR,
                            )
                    if final:
                        # per-chunk copy+store chains shorten the kernel
                        # tail: the last store only waits on a 512-col copy
                        for i in range(2):
                            copy_out(ot[:, 2 * jo + i, :], ps[:, i, :],
                                     force=("act" if i == 0 else "dve"))
                            (nc.scalar if i == 0 else nc.sync).dma_start(
                                our[:, 2 * jo + i, :], ot[:, 2 * jo + i, :])
                    else:
                        copy_out(ot[:, 2 * jo:2 * jo + 2, :], ps[:])
                        if jo == 1:
                            charge("pool", 0, fixed=994 + 0.34 * 256)
                        (nc.gpsimd if jo == 1 else nc.sync).dma_start(
                            our[:, 2 * jo:2 * jo + 2, :],
                            ot[:, 2 * jo:2 * jo + 2, :],
                        )

                return [lambda jo=jo: group(jo) for jo in range(HJ)]

            # Fine-grained depth-4 software pipeline. Iteration i emits
            # sample i's projections (A=P1T, B=Vc), sample i-1's attention
            # (C=scores+exp, D=zsum, E=norm), sample i-2's PV (F=PV+tanh),
            # and sample i-3's FC1 (G+store). G consumes ht produced a FULL
            # iteration earlier, so the tanh->psum-recycle chain never
            # stalls PE; every psum-pool reuse has multiple microseconds of
            # slack. G interleaves with A early (its inputs are long
            # ready); F sits late so its tanh drains during B and the next
            # iteration's C/A.
            state = {}   # sample index -> dict of live tiles/thunks

            def emit(th, label=None):
                if th is not None:
                    if label is not None:
                        _STAGE_MARKS.append((len(nc.inst_map), label))
                    th()

            prefetched = {0: sample0}
            for i in range(BPC + 3):
                if i + 1 < BPC:
                    prefetched[i + 1] = load_sample(i + 1)
                cur = None
                if i < BPC:
                    loaded = prefetched.pop(i)
                    cur = {"b": i, "loaded": loaded}
                    p1, cur["A"] = stage_a(i, loaded, j_outer=(i == 0))
                    vc, cur["B"] = stage_b(i, loaded)
                    cur["p1"], cur["vc"] = p1, vc
                mid = state.get(i - 1)   # sample doing attention this round
                if mid is not None:
                    et, mid["C"] = stage_c(mid["b"], mid["loaded"], mid["p1"])
                    mid["et"] = et
                old = state.get(i - 2)   # sample doing PV this round
                old2 = state.get(i - 3)  # sample finishing this round

                A = cur["A"] if cur else [None] * HJ
                Bg = cur["B"] if cur else [None] * SC
                Cg = mid["C"] if mid else [None] * SJ
                if old is not None:
                    ht, fthunks = stage_f(old["b"], old["vc"], old["et"])
                    old["ht"] = ht
                else:
                    fthunks = [None] * HJ
                gthunks = (stage_g(old2["b"], old2["ht"],
                                   final=(old2["b"] == BPC - 1))
                           if old2 is not None else [None] * HJ)

                emit(Cg[0], "C0")
                emit(Cg[1], "C1")
                emit(A[0], "A0")
                emit(gthunks[0], "G0")
                emit(A[1], "A1")
                if mid is not None:
                    _STAGE_MARKS.append((len(nc.inst_map), "D"))
                    mid["ps_z"] = zsum(mid["et"], mid["b"])
                    _STAGE_MARKS.append((len(nc.inst_map), "E"))
                    zb_norm(mid["et"], mid["ps_z"], mid["b"])
                emit(gthunks[1], "G1")
                emit(A[2], "A2")
                emit(gthunks[2], "G2")
                emit(fthunks[0], "F0")
                emit(Bg[0], "B0")
                emit(Bg[1], "B1")
                emit(fthunks[1], "F1")
                emit(Bg[2], "B2")
                emit(fthunks[2], "F2")
                emit(Bg[3], "B3")

                if old2 is not None:
                    del state[i - 3]
                if cur is not None:
                    state[i] = cur

    nc.finalize()
    return nc


def _spans_for(divide_pos):
    """Conservative per-slot q-spans. Samples are rank-sorted by divide_pos
    and dealt rank r -> (core r % 8, slot r // 8), so the 8 samples sharing
    a slot have adjacent d values; spans from the slot's [dmin, dmax] are
    nearly exact. Returns (assign[slot, core] -> orig index, spans)."""
    order = np.argsort(divide_pos, kind="stable")
    assign = order.reshape(BPC, NCORES)
    ds = divide_pos[assign]
    spans = []
    for j in range(BPC):
        lo, hi = int(ds[j].min()), int(ds[j].max())
        cs = []
        for c in range(SC):
            if P * (c + 1) <= lo:
                cs.append((lo, S))        # pure-query chunk: attends [d, S)
            elif P * c >= hi:
                cs.append((0, hi))        # pure-context chunk: attends [0, d)
            else:
                cs.append((0, S))
        fs = []
        for p_ in range(SJ):
            if 2 * P * (p_ + 1) <= lo:
                fs.append((lo, S))
            elif 2 * P * p_ >= hi:
                fs.append((0, hi))
            else:
                fs.append((0, S))
        spans.append((tuple(cs), tuple(fs)))
    return assign, tuple(spans)


def _get_nc(spans=None):
    if spans is None and "last" in _cache:
        return _cache["last"]
    key = ("nc", spans)
    if key not in _cache:
        _cache[key] = _build_program(spans)
    _cache["last"] = _cache[key]
    return _cache[key]


def kernel(**inputs):
    from concourse.bass_utils import run_bass_kernel_spmd

    x = np.asarray(inputs["x"], dtype=np.float32)            # [B,S,H]
    mask = np.asarray(inputs["mask"], dtype=np.float32)      # [B,S]
    divide_pos = np.asarray(inputs["divide_pos"]).astype(np.int64)  # [B]
    Wq = np.asarray(inputs["Wq"], dtype=np.float32)
    bq = np.asarray(inputs["bq"], dtype=np.float32)
    Wk = np.asarray(inputs["Wk"], dtype=np.float32)
    bk = np.asarray(inputs["bk"], dtype=np.float32)
    Wv = np.asarray(inputs["Wv"], dtype=np.float32)
    bv = np.asarray(inputs["bv"], dtype=np.float32)
    W0 = np.asarray(inputs["W0"], dtype=np.float32)
    b0 = np.asarray(inputs["b0"], dtype=np.float32)
    W1 = np.asarray(inputs["W1"], dtype=np.float32)
    b1 = np.asarray(inputs["b1"], dtype=np.float32)

    bf = ml_dtypes.bfloat16
    f8 = ml_dtypes.float8_e4m3
    f85 = ml_dtypes.float8_e5m2

    # ---- host-side fusion + prep ----
    Bsc = (Wq.T @ Wk) * RS                # scores core: x @ Bsc @ x.T
    Wc = W0 @ Wv                          # fused V.W0
    b0p = (b0 + W0 @ bv).astype(np.float32)
    u = (x @ (Wk.T @ bq)) * (RS * SB_OUT)     # [B,S] k-side bias term (x64)
    vq = (x @ (Wq.T @ bk)) * (RS * SB_OUT)    # [B,S] q-side bias term (x64)
    c = float(bq @ bk) * RS * SB_OUT

    xT = np.ascontiguousarray(x.transpose(0, 2, 1)).astype(f8)   # [B,H,S]
    Bsc8 = np.ascontiguousarray(Bsc * WS_B).astype(f8)           # layout [h, o]
    WcT8 = np.ascontiguousarray(Wc.T * WS_C).astype(f8)
    W1T8 = np.ascontiguousarray(W1.T * WS_1).astype(f8)

    # rank-4 mask/bias factors per sample (all x64 to match P1T scaling)
    pos = np.arange(S)
    isq = (pos[None, :] < divide_pos[:, None]).astype(np.float32)     # [B,S]
    rowQ = np.where(isq > 0, NEG, np.clip(mask * SB_OUT, NEG, None))  # [B,S]
    rowC = np.where(isq > 0, 0.0, NEG)                                # [B,S]
    ones = np.ones((B, S), np.float32)
    # rows r=2b+a laid out [a(partition), b(free pair)]: DR contracts (a,b)
    l4 = np.stack([rowQ, rowC, u + c, ones], axis=1).astype(f85)      # [B,4,S]
    r4 = np.stack([isq, 1.0 - isq, ones, vq], axis=1).astype(f85)     # [B,4,S]
    l4 = l4.reshape(B, 2, 2, S).transpose(0, 2, 1, 3)                 # [B,2,2,S]
    r4 = r4.reshape(B, 2, 2, S).transpose(0, 2, 1, 3)
    m4 = np.concatenate([l4, r4], axis=2).copy()                      # [B,2,4,S]

    # rank-sort samples by divide_pos across (slot, core) so each slot's
    # 8 cross-core samples share tight conservative attention spans; the
    # single shared program is specialized to those spans.
    assign, spans = _spans_for(divide_pos)
    nc = _get_nc(spans)
    in_maps = []
    for cid in range(NCORES):
        idx = assign[:, cid]  # orig sample index per slot
        in_maps.append({
            "xT": xT[idx],
            "Bsc": Bsc8, "WcT": WcT8, "W1T": W1T8, "b0p": b0p,
            "m4": m4[idx],
        })

    res = run_bass_kernel_spmd(nc, in_maps, core_ids=list(range(NCORES)))
    outT = np.empty((B, H, S), dtype=np.float32)
    for cid in range(NCORES):
        outT[assign[:, cid]] = np.asarray(
            res.results[cid]["outT"], dtype=np.float32)
    out = outT.transpose(0, 2, 1) * np.float32(1.0 / WS_1) + b1 + x
    return out.astype(np.float32)

